# revision 11
# baseline (speedup 1.0000x reference)
"""DGCNN (nn_DGCNNCls) Trainium2 Bass kernel.

Data-parallel over batch: 16 samples -> 8 NeuronCores x 2 samples, zero
communication. Per sample everything stays on-chip in a channels-on-partitions
(transposed) layout.

Per EdgeConv layer (C -> O):
  scores  S[n,j] = 2<x_n,x_j> - ||x_j||^2  on PE in fp32 (the row-constant
          ||x_n||^2 term does not change per-row top-k and is dropped; the
          -||x_j||^2 term is fused in as one extra contraction row).
  top-k   exact k=20: 3 rounds of DVE max8 / max_index / match_replace.
  gather  EdgeConv is refactored as h[n,k,o] = A[idx[n,k],o] + B[n,o] with
          A = X W1^T diag|g|, B = X (W2-W1)^T diag(g) (+bn bias), where
          W=[W1 W2] splits the 1x1 conv and g is the folded BN scale. The
          |g| / sign split lets max over neighbors commute with BN.
          A^T ([O,N], channels on partitions) is gathered along the free
          axis with GPSIMD ap_gather; the wrapped 16-partition index lists
          are exactly rows of a PE transpose of the per-tile index block,
          so no index relayout is needed (plane i <-> partition i).
  reduce  DVE segmented max over k, then y = leaky(sign*G + B).

conv5 (512->1024), global max+sum pooling (mean folded into lin1 weights),
and the 3-layer head run on PE in fp32.
"""

import numpy as np

import concourse.bass as bass
import concourse.bacc as bacc
import concourse.mybir as mybir
from concourse.bass import ts
from concourse.masks import make_identity
from concourse.tile import TileContext
from concourse import library_config

dt = mybir.dt
AF = mybir.ActivationFunctionType
ALU = mybir.AluOpType

N = 2048
NT = 16            # 128-row tiles per sample
K = 20
EPS = 1e-5
LAYERS = [(3, 64), (64, 64), (64, 128), (128, 256)]
NEG_BIG = -3.0e38
NCHUNK = 16        # gather chunks per layer
CN = N // NCHUNK   # points per gather chunk


class _PsumMux:
    def __init__(self, big, tr):
        self.big = big
        self.tr = tr

    def tile(self, shape, dtype, tag):
        pool = self.big if tag == "big" else self.tr
        return pool.tile(shape, dtype, tag=tag, name=f"ps_{tag}")


def _leaky_inplace(nc, ap):
    # y = max(0.2*u, u)
    nc.vector.scalar_tensor_tensor(out=ap, in0=ap, scalar=0.2, in1=ap,
                                   op0=ALU.mult, op1=ALU.max)


def _edge_conv_layer(nc, p1, p2, psum, li, C, O, Xt, Y_parts,
                     wA_sb, wB_sb, sg_sb, bt_sb, ident):
    """One EdgeConv layer for one sample.

    Xt: [C, N] SBUF AP (partitions 0..C-1). Y_parts: list of [oc, N] APs
    (partitions 0..oc-1) for the output, one per 128-wide O tile.
    """
    aug = (C + 1 <= 128) and (C % 32 == 0)

    # squares, then xx[j] = sum_c Xt[c,j]^2 via PE ones-matvec
    SQ = p1.tile([C, N], dt.float32, tag="scr8")
    nc.scalar.activation(SQ, Xt, AF.Square)
    onesC = p1.tile([C, 1], dt.float32, tag="onesC")
    nc.vector.memset(onesC, 1.0)
    ps_xx = psum.tile([1, N], dt.float32, tag="big")
    for q in range(4):
        nc.tensor.matmul(ps_xx[:, ts(q, 512)], onesC, SQ[:, ts(q, 512)],
                         start=True, stop=True)
    xxneg = p1.tile([1, N], dt.float32, tag="xxneg")
    nc.scalar.activation(xxneg, ps_xx, AF.Copy, scale=-1.0)

    if aug:
        XA = p1.tile([C + 1, N], dt.float32, tag="XA")
        RA = p1.tile([C + 1, N], dt.float32, tag="RA")
        nc.sync.dma_start(out=XA[0:C, :], in_=Xt)
        nc.vector.memset(XA[C:C + 1, :], 1.0)
        nc.scalar.activation(RA[0:C, :], Xt, AF.Copy, scale=2.0)
        nc.sync.dma_start(out=RA[C:C + 1, :], in_=xxneg)
        lhsT_src, rhs_src = XA, RA
    else:
        RA = p1.tile([C, N], dt.float32, tag="RA")
        nc.scalar.activation(RA, Xt, AF.Copy, scale=2.0)
        ones_row = p1.tile([1, N], dt.float32, tag="XA")
        nc.vector.memset(ones_row, 1.0)
        lhsT_src, rhs_src = Xt, RA

    # A' and B' (transposed, channels on partitions)
    n_ot = (O + 127) // 128
    At, Bt = [], []
    for ot in range(n_ot):
        oc = min(128, O - ot * 128)
        ps = psum.tile([oc, N], dt.float32, tag="big")
        for q in range(4):
            nc.tensor.matmul(ps[:, ts(q, 512)],
                             wA_sb[:, ot * 128:ot * 128 + oc],
                             Xt[:, ts(q, 512)], start=True, stop=True)
        a_sb = p1.tile([oc, N], dt.float32, tag=f"At{ot}")
        nc.scalar.copy(a_sb, ps)
        At.append(a_sb)

        ps2 = psum.tile([oc, N], dt.float32, tag="big")
        for q in range(4):
            nc.tensor.matmul(ps2[:, ts(q, 512)],
                             wB_sb[:, ot * 128:ot * 128 + oc],
                             Xt[:, ts(q, 512)], start=True, stop=True)
        b_sb = p1.tile([oc, N], dt.float32, tag=f"Bt{ot}")
        nc.scalar.activation(b_sb, ps2, AF.Identity,
                             bias=bt_sb[0:oc, ot:ot + 1])
        Bt.append(b_sb)

    # score tiles + top-k -> idxT_all [32, N] int16 (plane-major)
    idxT_all = p1.tile([32, N], dt.int16, tag="idxT")
    for t in range(NT):
        ps_s = psum.tile([128, N], dt.float32, tag="big")
        for q in range(4):
            if aug:
                nc.tensor.matmul(ps_s[:, ts(q, 512)], lhsT_src[:, ts(t, 128)],
                                 rhs_src[:, ts(q, 512)], start=True, stop=True)
            else:
                nc.tensor.matmul(ps_s[:, ts(q, 512)], lhsT_src[:, ts(t, 128)],
                                 rhs_src[:, ts(q, 512)], start=True, stop=False)
                nc.tensor.matmul(ps_s[:, ts(q, 512)], ones_row[:, ts(t, 128)],
                                 xxneg[:, ts(q, 512)], start=False, stop=True)
        S_sb = p2.tile([128, N], dt.float32, tag="S_sb")
        nc.scalar.copy(S_sb, ps_s)

        MX = p1.tile([128, 8], dt.float32, tag="MX")
        MI = p1.tile([128, 24], dt.uint32, tag="MI")
        for r in range(3):
            nc.vector.max(out=MX, in_=S_sb)
            nc.vector.max_index(out=MI[:, r * 8:(r + 1) * 8], in_max=MX,
                                in_values=S_sb)
            if r < 2:
                nc.vector.match_replace(out=S_sb, in_to_replace=MX,
                                        in_values=S_sb, imm_value=NEG_BIG)

        idxF = p1.tile([128, 32], dt.float32, tag="idxF")
        nc.vector.tensor_copy(idxF[:, 0:24], MI)
        nc.vector.tensor_copy(idxF[:, 20:32],
                              idxF[:, 0:1].to_broadcast([128, 12]))
        ps_t = psum.tile([32, 128], dt.float32, tag="tr")
        nc.tensor.transpose(ps_t, idxF, ident)
        nc.vector.tensor_copy(idxT_all[:, ts(t, 128)], ps_t)

    # replicate wrapped index lists to every 16-partition group (DMA: it
    # crosses partitions)
    repA = p1.tile([128, N], dt.int16, tag="repA")
    repB = p1.tile([128, N], dt.int16, tag="repB")
    for g in range(8):
        nc.sync.dma_start(out=repA[g * 16:(g + 1) * 16, :],
                          in_=idxT_all[0:16, :])
        nc.sync.dma_start(out=repB[g * 16:(g + 1) * 16, :],
                          in_=idxT_all[16:32, :])

    # gather + aggregate + activation
    for ot in range(n_ot):
        oc = At[ot].shape[0]
        U = p1.tile([oc, N], dt.float32, tag="scr8")
        for c in range(NCHUNK):
            ga = p1.tile([oc, CN * 16], dt.float32, tag="ga")
            nc.gpsimd.ap_gather(ga, At[ot], repA[0:oc, c * CN:(c + 1) * CN],
                                channels=oc, num_elems=N, d=1,
                                num_idxs=CN * 16)
            gb = p1.tile([oc, CN * 16], dt.float32, tag="gb")
            nc.gpsimd.ap_gather(gb, At[ot], repB[0:oc, c * CN:(c + 1) * CN],
                                channels=oc, num_elems=N, d=1,
                                num_idxs=CN * 16)
            ra = p1.tile([oc, CN], dt.float32, tag="ra")
            nc.vector.tensor_reduce(ra,
                                    ga.rearrange("p (n k) -> p n k", k=16),
                                    axis=mybir.AxisListType.X, op=ALU.max)
            rb = p1.tile([oc, CN], dt.float32, tag="rb")
            nc.vector.tensor_reduce(
                rb, gb.rearrange("p (n k) -> p n k", k=16)[:, :, 0:4],
                axis=mybir.AxisListType.X, op=ALU.max)
            nc.vector.tensor_max(U[:, c * CN:(c + 1) * CN], ra, rb)
        # y = leaky(sign * G + B)
        yt = Y_parts[ot]
        nc.vector.scalar_tensor_tensor(out=yt, in0=U,
                                       scalar=sg_sb[0:oc, ot:ot + 1],
                                       in1=Bt[ot], op0=ALU.mult, op1=ALU.add)
        _leaky_inplace(nc, yt)


def build_nc(n_samples=2):
    nc = bacc.Bacc()
    x_in = nc.dram_tensor("x", [n_samples, 3, N], dt.float32,
                          kind="ExternalInput")
    wA_d, wB_d, sg_d, bt_d = {}, {}, {}, {}
    for li, (C, O) in enumerate(LAYERS):
        n_ot = (O + 127) // 128
        wA_d[li] = nc.dram_tensor(f"wA{li}", [C, O], dt.float32,
                                  kind="ExternalInput")
        wB_d[li] = nc.dram_tensor(f"wB{li}", [C, O], dt.float32,
                                  kind="ExternalInput")
        sg_d[li] = nc.dram_tensor(f"sg{li}", [128, n_ot], dt.float32,
                                  kind="ExternalInput")
        bt_d[li] = nc.dram_tensor(f"bt{li}", [128, n_ot], dt.float32,
                                  kind="ExternalInput")
    w5_d = nc.dram_tensor("w5", [128, 5, 1024], dt.float32,
                          kind="ExternalInput")
    b5_d = nc.dram_tensor("b5", [128, 8], dt.float32, kind="ExternalInput")
    l1_d = nc.dram_tensor("l1w", [128, 16, 512], dt.float32,
                          kind="ExternalInput")
    b6_d = nc.dram_tensor("b6", [128, 4], dt.float32, kind="ExternalInput")
    l2_d = nc.dram_tensor("l2w", [128, 4, 256], dt.float32,
                          kind="ExternalInput")
    b7_d = nc.dram_tensor("b7", [128, 2], dt.float32, kind="ExternalInput")
    l3_d = nc.dram_tensor("l3w", [128, 2, 40], dt.float32,
                          kind="ExternalInput")
    b3_d = nc.dram_tensor("b3", [40, 1], dt.float32, kind="ExternalInput")
    out_d = nc.dram_tensor("out", [n_samples, 40], dt.float32,
                           kind="ExternalOutput")

    with TileContext(nc) as tc:
        with tc.tile_pool(name="w", bufs=1) as wpool, \
             tc.tile_pool(name="p1", bufs=1) as p1, \
             tc.tile_pool(name="p2", bufs=2) as p2, \
             tc.tile_pool(name="psum_big", bufs=1, space="PSUM") as psum_big, \
             tc.tile_pool(name="psum_tr", bufs=2, space="PSUM") as psum_tr:

            psum = _PsumMux(psum_big, psum_tr)

            ident = wpool.tile([128, 128], dt.float32, tag="ident")
            make_identity(nc, ident)

            wA_sb, wB_sb, sg_sb, bt_sb = {}, {}, {}, {}
            for li, (C, O) in enumerate(LAYERS):
                n_ot = (O + 127) // 128
                wA_sb[li] = wpool.tile([C, O], dt.float32, tag=f"wA{li}", name=f"wA{li}s")
                nc.sync.dma_start(out=wA_sb[li], in_=wA_d[li][:, :])
                wB_sb[li] = wpool.tile([C, O], dt.float32, tag=f"wB{li}", name=f"wB{li}s")
                nc.sync.dma_start(out=wB_sb[li], in_=wB_d[li][:, :])
                sg_sb[li] = wpool.tile([128, n_ot], dt.float32, tag=f"sg{li}", name=f"sg{li}s")
                nc.sync.dma_start(out=sg_sb[li], in_=sg_d[li][:, :])
                bt_sb[li] = wpool.tile([128, n_ot], dt.float32, tag=f"bt{li}", name=f"bt{li}s")
                nc.sync.dma_start(out=bt_sb[li], in_=bt_d[li][:, :])
            b5_sb = wpool.tile([128, 8], dt.float32, tag="b5")
            nc.sync.dma_start(out=b5_sb, in_=b5_d[:, :])
            b6_sb = wpool.tile([128, 4], dt.float32, tag="b6")
            nc.sync.dma_start(out=b6_sb, in_=b6_d[:, :])
            b7_sb = wpool.tile([128, 2], dt.float32, tag="b7")
            nc.sync.dma_start(out=b7_sb, in_=b7_d[:, :])
            l2_sb = wpool.tile([128, 4, 256], dt.float32, tag="l2")
            nc.sync.dma_start(out=l2_sb, in_=l2_d[:, :, :])
            l3_sb = wpool.tile([128, 2, 40], dt.float32, tag="l3")
            nc.sync.dma_start(out=l3_sb, in_=l3_d[:, :, :])
            b3_sb = wpool.tile([40, 1], dt.float32, tag="b3")
            nc.sync.dma_start(out=b3_sb, in_=b3_d[:, :])

            for s in range(n_samples):
                Y1 = p1.tile([64, N], dt.float32, tag=f"Y1_{s}")
                Y2 = p1.tile([64, N], dt.float32, tag=f"Y2_{s}")
                H3 = p1.tile([128, N], dt.float32, tag="H3")
                H4a = p1.tile([128, N], dt.float32, tag="H4a")
                H4b = p1.tile([128, N], dt.float32, tag="H4b")

                X0 = p1.tile([3, N], dt.float32, tag="ga")
                nc.sync.dma_start(out=X0, in_=x_in[s])

                _edge_conv_layer(nc, p1, p2, psum, 0, 3, 64, X0, [Y1],
                                 wA_sb[0], wB_sb[0], sg_sb[0], bt_sb[0], ident)
                _edge_conv_layer(nc, p1, p2, psum, 1, 64, 64, Y1, [Y2],
                                 wA_sb[1], wB_sb[1], sg_sb[1], bt_sb[1], ident)
                _edge_conv_layer(nc, p1, p2, psum, 2, 64, 128, Y2, [H3],
                                 wA_sb[2], wB_sb[2], sg_sb[2], bt_sb[2], ident)
                _edge_conv_layer(nc, p1, p2, psum, 3, 128, 256, H3,
                                 [H4a, H4b],
                                 wA_sb[3], wB_sb[3], sg_sb[3], bt_sb[3], ident)

                # conv5 + pooling
                kchunks = [(Y1, 0, 64), (Y2, 64, 64), (H3, 128, 128),
                           (H4a, 256, 128), (H4b, 384, 128)]
                gmax = p1.tile([128, 8], dt.float32, tag=f"gmax_{s}")
                gsum = p1.tile([128, 8], dt.float32, tag=f"gsum_{s}")
                for ot in range(8):
                    w5s = p1.tile([128, 5, 128], dt.float32, tag="w5s")
                    nc.sync.dma_start(out=w5s, in_=w5_d[:, :, ts(ot, 128)])
                    ps5 = psum.tile([128, N], dt.float32, tag="big")
                    for q in range(4):
                        for ki, (ht, kbase, kc) in enumerate(kchunks):
                            nc.tensor.matmul(
                                ps5[:, ts(q, 512)], w5s[0:kc, ki, :],
                                ht[:, ts(q, 512)],
                                start=(ki == 0), stop=(ki == len(kchunks) - 1))
                    h5 = p1.tile([128, N], dt.float32, tag="scr8")
                    nc.scalar.activation(h5, ps5, AF.Identity,
                                         bias=b5_sb[:, ot:ot + 1])
                    _leaky_inplace(nc, h5)
                    nc.vector.tensor_reduce(gmax[:, ot:ot + 1], h5,
                                            axis=mybir.AxisListType.X,
                                            op=ALU.max)
                    nc.vector.tensor_reduce(gsum[:, ot:ot + 1], h5,
                                            axis=mybir.AxisListType.X,
                                            op=ALU.add)

                # head (g chunks: 0..7 = gmax cols, 8..15 = gsum cols; the
                # 1/2048 mean factor is folded into l1 rows host-side)
                h6 = p1.tile([128, 4], dt.float32, tag=f"h6_{s}")
                for ot in range(4):
                    hw = p1.tile([128, 16, 128], dt.float32, tag="hw")
                    nc.sync.dma_start(out=hw, in_=l1_d[:, :, ts(ot, 128)])
                    ps6 = psum.tile([128, 1], dt.float32, tag="tr")
                    for kc in range(16):
                        gsrc = gmax[:, kc:kc + 1] if kc < 8 \
                            else gsum[:, kc - 8:kc - 7]
                        nc.tensor.matmul(ps6, hw[:, kc, :], gsrc,
                                         start=(kc == 0), stop=(kc == 15))
                    nc.scalar.activation(h6[:, ot:ot + 1], ps6, AF.Identity,
                                         bias=b6_sb[:, ot:ot + 1])
                    _leaky_inplace(nc, h6[:, ot:ot + 1])
                h7 = p1.tile([128, 2], dt.float32, tag=f"h7_{s}")
                for ot in range(2):
                    ps7 = psum.tile([128, 1], dt.float32, tag="tr")
                    for kc in range(4):
                        nc.tensor.matmul(ps7, l2_sb[:, kc, ts(ot, 128)],
                                         h6[:, kc:kc + 1],
                                         start=(kc == 0), stop=(kc == 3))
                    nc.scalar.activation(h7[:, ot:ot + 1], ps7, AF.Identity,
                                         bias=b7_sb[:, ot:ot + 1])
                    _leaky_inplace(nc, h7[:, ot:ot + 1])
                ps8 = psum.tile([40, 1], dt.float32, tag="tr")
                for kc in range(2):
                    nc.tensor.matmul(ps8, l3_sb[0:128, kc, :],
                                     h7[:, kc:kc + 1],
                                     start=(kc == 0), stop=(kc == 1))
                logit = p1.tile([40, 1], dt.float32, tag=f"logit_{s}")
                nc.scalar.activation(logit, ps8, AF.Identity, bias=b3_sb)
                nc.sync.dma_start(
                    out=out_d[s:s + 1, :].rearrange("a b -> b a"), in_=logit)
    nc.finalize()
    return nc


def _fold_bn(p):
    g, b, m, v = [np.asarray(t, np.float32) for t in p]
    gt = (g / np.sqrt(v + EPS)).astype(np.float32)
    bt = (b - m * gt).astype(np.float32)
    return gt, bt


def _pad_cols(a, n_ot):
    """(O,) -> [128, n_ot] column-per-otile layout."""
    O = a.shape[0]
    out = np.zeros((128, n_ot), np.float32)
    for ot in range(n_ot):
        oc = min(128, O - ot * 128)
        out[0:oc, ot] = a[ot * 128:ot * 128 + oc]
    return out


def _prep_inputs(params):
    d = {}
    for li, (C, O) in enumerate(LAYERS):
        n_ot = (O + 127) // 128
        w = np.asarray(params[f'conv{li + 1}_w'], np.float32)   # (O, 2C)
        gt, bt = _fold_bn(params[f'bn{li + 1}'])
        W1 = w[:, :C]
        W2 = w[:, C:]
        sg = np.sign(gt).astype(np.float32)
        sg[sg == 0] = 1.0
        d[f"wA{li}"] = np.ascontiguousarray(W1.T * np.abs(gt)[None, :])
        d[f"wB{li}"] = np.ascontiguousarray((W2 - W1).T * gt[None, :])
        d[f"sg{li}"] = _pad_cols(sg, n_ot)
        d[f"bt{li}"] = _pad_cols(bt, n_ot)
    g5, b5 = _fold_bn(params['bn5'])
    w5 = np.asarray(params['conv5_w'], np.float32)          # (1024, 512)
    w5t = (w5 * g5[:, None]).T                              # (512, 1024)
    w5p = np.zeros((128, 5, 1024), np.float32)
    w5p[0:64, 0] = w5t[0:64]
    w5p[0:64, 1] = w5t[64:128]
    w5p[:, 2] = w5t[128:256]
    w5p[:, 3] = w5t[256:384]
    w5p[:, 4] = w5t[384:512]
    d["w5"] = np.ascontiguousarray(w5p)
    d["b5"] = np.ascontiguousarray(b5.reshape(8, 128).T)
    g6, b6 = _fold_bn(params['bn6'])
    l1 = np.asarray(params['lin1_w'], np.float32) * g6[:, None]  # (512, 2048)
    l1 = l1.T.copy()                                        # (2048, 512)
    l1[1024:, :] *= np.float32(1.0 / N)                     # fold the mean
    d["l1w"] = np.ascontiguousarray(l1.reshape(16, 128, 512).transpose(1, 0, 2))
    d["b6"] = np.ascontiguousarray(b6.reshape(4, 128).T)
    g7, b7 = _fold_bn(params['bn7'])
    l2 = (np.asarray(params['lin2_w'], np.float32) * g7[:, None]).T  # (512,256)
    d["l2w"] = np.ascontiguousarray(l2.reshape(4, 128, 256).transpose(1, 0, 2))
    l2b = np.asarray(params['lin2_b'], np.float32)
    d["b7"] = np.ascontiguousarray((g7 * l2b + b7).reshape(2, 128).T)
    l3 = np.asarray(params['lin3_w'], np.float32).T         # (256, 40)
    d["l3w"] = np.ascontiguousarray(l3.reshape(2, 128, 40).transpose(1, 0, 2))
    d["b3"] = np.asarray(params['lin3_b'], np.float32)[:, None].copy()
    return d


_NC_CACHE = {}
_LAST_RESULT = {}


def kernel(x, params):
    from concourse.bass_utils import run_bass_kernel_spmd

    x = np.asarray(x, np.float32)
    B = x.shape[0]
    n_cores = 8
    spc = B // n_cores
    if spc not in _NC_CACHE:
        _NC_CACHE[spc] = build_nc(spc)
    nc = _NC_CACHE[spc]

    common = _prep_inputs(params)
    in_maps = []
    for c in range(n_cores):
        m = dict(common)
        m["x"] = np.ascontiguousarray(x[c * spc:(c + 1) * spc])
        in_maps.append(m)
    import os
    trace = bool(os.environ.get("BASS_TRACE"))
    res = run_bass_kernel_spmd(nc, in_maps, list(range(n_cores)), trace=trace)
    _LAST_RESULT["res"] = res
    out = np.concatenate([res.results[c]["out"] for c in range(n_cores)], 0)
    return out.astype(np.float32)
